# revision 8
# baseline (speedup 1.0000x reference)
"""CapsNet dynamic-routing kernel for TRN2, 8 NeuronCores, data-parallel over batch.

Routing math is fully batch-local; the kernel is a per-batch pipeline hidden
under the u-vec DMA stream:

  host: iter-0 (softmax(0) is uniform) -> outputs0, w20 = W @ outputs0 shipped
  chip: per batch b:  b1 = w20 @ u^T -> softmax -> v1 = c1^T u   (as UT/U land)
        per group:    pre1 = v1 @ W -> squash -> w21 = W @ out1  (T-pair pipelined)
        per batch:    b2 = w21 @ u^T -> softmax -> v2
        per group:    pre2 -> squash -> output DMA

Perf notes vs the earlier revision:
  - squash scale = exp(-0.5*ln(|pre|^2)): Ln and Exp live in the same act
    table set (natural_log_exp_and_others), so ONE act-table load for the
    whole kernel instead of 8 Exp<->Sqrt swaps (~10us + serialization).
    |pre|^2 is O(100) here so the reference's 1e-7 eps is numerically inert.
  - PE HAM warm-up: ~4us of junk matmuls at t~1.5us flips the PE clock gate
    to 2.4GHz before real work; keep-warm matmuls anchored to the W16/WT16
    DMA halves bridge the two >3.4us PE-idle windows in the early stream so
    the PE never re-throttles to 1.2GHz.
  - DMA order: W16/WT16 stream *between* early batches (needed by pre1A/w2A
    ~30us in), batches 3..7 stream back-to-back so the tail (which is gated
    by the last batch's arrival) starts as early as possible.
  - PSUM->SBUF copies moved off the scalar queue (DVE), scalar runs only Exp
    softmaxes + Ln/Exp squash scales + w2 gathers.
fp16 operands / fp32 accumulation; inputs host-packed partition-major.
"""

import numpy as np

ROUTINGS = 3
NC_CAP = 32
DC = 64
EPS = 1e-7
N_CORES = 8
B, N_IN, D_IN = 64, 1024, 512
B_LOC = B // N_CORES  # 8

_cached = {}


def _build_program():
    import concourse.bass as bass
    import concourse.tile as tile
    from concourse import bacc, mybir
    from concourse.hw_specs import get_activation_tables

    f16 = mybir.dt.float16
    f32 = mybir.dt.float32
    ADD = mybir.AluOpType.add
    AX = mybir.AxisListType.X
    AF = mybir.ActivationFunctionType

    nc = bacc.Bacc("TRN2", target_bir_lowering=False, debug=False,
                   num_devices=N_CORES)
    act_tabs = list(get_activation_tables(nc.m.arch).keys())
    LNEXP_SET = act_tabs.index("natural_log_exp_and_others")

    # host-packed, SBUF-native layouts (partition dim first, contiguous rows)
    w16_d = nc.dram_tensor("w16", [128, 4, NC_CAP * DC], f16, kind="ExternalInput").ap()
    wt16_d = nc.dram_tensor("wt16", [128, 16, D_IN], f16, kind="ExternalInput").ap()
    w20t_d = nc.dram_tensor("w20t", [128, 4, B_LOC, NC_CAP], f16, kind="ExternalInput").ap()
    ut_d = nc.dram_tensor("ut16", [B_LOC, 128, 4, N_IN], f16, kind="ExternalInput").ap()
    u_d = nc.dram_tensor("u16", [B_LOC, 128, 8, D_IN], f16, kind="ExternalInput").ap()
    ident_d = nc.dram_tensor("ident", [128, 128], f16, kind="ExternalInput").ap()
    outA_d = nc.dram_tensor("outA", [128, 8 * DC], f32, kind="ExternalOutput").ap()
    outB_d = nc.dram_tensor("outB", [128, 8 * DC], f32, kind="ExternalOutput").ap()
    out_drams = [outA_d, outB_d]

    with tile.TileContext(nc) as tc:
        with (
            tc.tile_pool(name="big", bufs=1) as big,
            tc.tile_pool(name="work", bufs=1) as work,
            tc.tile_pool(name="sbE", bufs=2) as sbE,
            tc.tile_pool(name="sbP", bufs=2) as sbP,
            tc.tile_pool(name="sbO", bufs=4) as sbO,
            tc.tile_pool(name="psB", bufs=2, space="PSUM") as psB,
            tc.tile_pool(name="psV", bufs=2, space="PSUM") as psV,
            tc.tile_pool(name="psPre", bufs=2, space="PSUM") as psPre,
            tc.tile_pool(name="psT", bufs=1, space="PSUM") as psT,
            tc.tile_pool(name="psW2", bufs=1, space="PSUM") as psW2,
        ):
            U = big.tile([128, B_LOC, 8, D_IN], f16, tag="U")      # (i%128),(b),(i//128),(k)
            UT = big.tile([128, B_LOC, 4, N_IN], f16, tag="UT")    # (k%128),(b),(k//128),(i)
            W16 = big.tile([128, 4, NC_CAP * DC], f16, tag="W16")  # (k%128),(k//128),(n d)
            WT16 = big.tile([128, 16, D_IN], f16, tag="WT16")      # (tau d),(m g),(k)
            W20T = big.tile([128, 4, B_LOC, NC_CAP], f16, tag="W20T")
            IDENT = work.tile([128, 128], f16, tag="IDENT")
            JUNK = work.tile([128, 256], f16, tag="JUNK")

            # per-group tiles (A: batches 0-3, B: 4-7) to avoid false WARs
            vT = [work.tile([128, 4, 4, NC_CAP], f16, tag=f"vT{g}",
                            name=f"vT{g}") for g in range(2)]
            w2T = [work.tile([128, 4, 4, NC_CAP], f16, tag=f"w2T{g}",
                             name=f"w2T{g}") for g in range(2)]
            c_sb = [work.tile([128, 4, 8, NC_CAP], f16, tag=f"c{g}",
                              name=f"c{g}") for g in range(2)]
            outT = [work.tile([128, 4, 128], f16, tag=f"outT{g}",
                              name=f"outT{g}") for g in range(2)]
            L_sb = [[work.tile([128, 4, 2, 4], f16, tag=f"L{g}_{m}",
                               name=f"L{g}_{m}") for m in range(4)]
                    for g in range(2)]
            outp32 = [work.tile([128, 8, DC], f32, tag=f"outp32_{g}",
                                name=f"outp32_{g}") for g in range(2)]
            z_sb = work.tile([128, B_LOC, 8], f32, tag="z")
            r_sb = work.tile([128, B_LOC, 8], f32, tag="r")

            # ---- single act-table load: Ln+Exp (+Copy) in one set ----
            nc.scalar.add_instruction(mybir.InstLoadActFuncSet(
                name=f"I-{nc.next_id()}", act_func_set_id=LNEXP_SET))
            nc.gpsimd.memset(JUNK[:], 0.5)
            for g in range(2):
                for m in range(4):
                    nc.gpsimd.memset(L_sb[g][m][:], 0.0)

            # ---- PE HAM warm-up: ~4us of junk matmuls from t~1.5us ----
            # (shares the psW2 "w2pn" slot: w2 only starts ~36us in, long
            # after the last keep-warm has retired, so the WAR is free)
            dummy_ps = psW2.tile([128, 4, 16, 2, 4], f32, tag="w2pn")
            with nc.named_scope("ham_warmup"):
                for i in range(36):
                    nc.tensor.matmul(dummy_ps[:, i % 4], JUNK[:, 0:128],
                                     JUNK[:, 0:128],
                                     start=True, stop=True)

            def keep_warm(rhs):
                # junk matmul whose rhs is a just-landed DMA slice: fires the
                # moment the data arrives, bridging a PE-idle window so the
                # HAM clock gate never re-throttles
                nc.tensor.matmul(dummy_ps[:, 0], JUNK[:, 0:128], rhs,
                                 start=True, stop=True)

            # ---- DMA queue: order == consumption order of the pipeline ----
            nc.sync.dma_start(W20T[:], w20t_d[:])
            nc.sync.dma_start(UT[:, 0], ut_d[0])
            nc.sync.dma_start(U[:, 0], u_d[0])
            nc.sync.dma_start(W16[:, 0:2], w16_d[:, 0:2])
            nc.sync.dma_start(W16[:, 2:4], w16_d[:, 2:4])
            nc.sync.dma_start(UT[:, 1], ut_d[1])
            nc.sync.dma_start(U[:, 1], u_d[1])
            nc.sync.dma_start(WT16[:, 0:8], wt16_d[:, 0:8])
            nc.sync.dma_start(WT16[:, 8:16], wt16_d[:, 8:16])
            nc.sync.dma_start(UT[:, 2], ut_d[2])
            nc.sync.dma_start(U[:, 2], u_d[2])
            nc.sync.dma_start(IDENT[:], ident_d[:])
            for b in range(3, B_LOC):
                nc.sync.dma_start(UT[:, b], ut_d[b])
                nc.sync.dma_start(U[:, b], u_d[b])

            def bup(b, it):
                # b-logits for batch b: [i%128, t, n] = sum_k u^T chunks @ w2T
                src = W20T if it == 1 else w2T[b // 4]
                bl = b if it == 1 else b % 4
                with nc.named_scope(f"i{it}_bup{b}"):
                    b_ps = psB.tile([128, 8, NC_CAP], f32, tag="b_ps")
                    for t in range(8):
                        for j in range(4):
                            nc.tensor.matmul(
                                b_ps[:, t], UT[:, b, j, 128 * t:128 * t + 128],
                                src[:, j, bl, :], start=(j == 0), stop=(j == 3))
                    e_sb = sbE.tile([128, 8, NC_CAP], f16, tag="e_sb")
                    nc.scalar.activation(e_sb[:], b_ps[:], AF.Exp)
                    nc.vector.tensor_reduce(z_sb[:, b], e_sb[:], AX, ADD)
                    nc.vector.reciprocal(r_sb[:, b], z_sb[:, b])
                    nc.vector.tensor_mul(
                        c_sb[b // 4][:, b % 4], e_sb[:],
                        r_sb[:, b].broadcast_to((128, 8, NC_CAP)))

            def vmm(b, it):
                with nc.named_scope(f"i{it}_v{b}"):
                    vT_ps = psV.tile([128, 4, NC_CAP], f32, tag="vT_ps")
                    for j in range(4):
                        for t in range(8):
                            nc.tensor.matmul(
                                vT_ps[:, j], U[:, b, t, 128 * j:128 * j + 128],
                                c_sb[b // 4][:, b % 4, t, :],
                                start=(t == 0), stop=(t == 7))
                    nc.vector.tensor_copy(vT[b // 4][:, :, b % 4, :], vT_ps[:])

            def pre_piece(grp, m):
                # capsules n = 4T+g for T in {2m, 2m+1}; fresh PSUM tile per piece
                pp = psPre.tile([128, 2, DC], f32, tag="pre")
                for tl in range(2):
                    for g in range(4):
                        n = 4 * (2 * m + tl) + g
                        for j in range(4):
                            nc.tensor.matmul(
                                pp[32 * g:32 * g + 4, tl],
                                vT[grp][:, j, :, n],
                                W16[:, j, 64 * n:64 * n + 64],
                                start=(j == 0), stop=(j == 3),
                                tile_position=(0, 32 * g),
                            )
                return pp

            def squash_piece(pp, dst, dsl):
                # dst[:, dsl] = pp / sqrt(|pp|^2), norm over d per capsule;
                # 1/sqrt(x) computed as exp(-0.5*ln(x)) to stay in one table set
                pre_c = sbP.tile([128, 2, DC], f32, tag="pre_c")
                sq2 = sbP.tile([128, 2, DC], f32, tag="sq2")
                nrm = sbP.tile([128, 2], f32, tag="nrm")
                lgn = sbP.tile([128, 2], f32, tag="lgn")
                scl = sbP.tile([128, 2], f32, tag="scl")
                nc.vector.tensor_copy(pre_c[:], pp[:])
                nc.vector.tensor_mul(sq2[:], pre_c[:], pp[:])
                nc.vector.tensor_reduce(nrm[:], sq2[:], AX, ADD)
                nc.scalar.activation(lgn[:], nrm[:], AF.Ln)
                nc.scalar.activation(scl[:], lgn[:], AF.Exp, scale=-0.5)
                nc.vector.tensor_mul(dst[:, dsl], pp[:],
                                     scl[:].broadcast_to((128, 2, DC)))

            def w2_piece(grp, w2pn, m, o16):
                # transpose scaled outputs T-pair m -> outT[(tau d), (g c)],
                # mask into L, then contract d for this piece's capsule pairs
                tp = psT.tile([128, 128], f16, tag="tp")
                nc.tensor.transpose(
                    tp[:], o16[:].rearrange("p a b -> p (a b)"), IDENT[:])
                nc.vector.tensor_copy(outT[grp][:, m], tp[:])
                for tau in range(2):
                    nc.vector.tensor_copy(
                        L_sb[grp][m][64 * tau:64 * tau + 64, :, tau, :],
                        outT[grp][64 * tau:64 * tau + 64, m, :]
                        .rearrange("p (g c) -> p g c", g=4)[:, :, 0:4])
                for p in range(4 * m, 4 * m + 4):
                    for j in range(4):
                        nc.tensor.matmul(
                            w2pn[:, j, p], WT16[:, p, 128 * j:128 * j + 128],
                            L_sb[grp][m][:, p - 4 * m], start=True, stop=True)

            def w2_gather(grp, w2pn):
                w2v = w2T[grp][:].rearrange(
                    "p j b (m x g) -> p x j m g b", m=4, x=2, g=4)
                for tau in range(2):
                    for j in range(4):
                        nc.scalar.copy(
                            w2v[:, tau, j],
                            w2pn[:, j, :, tau].rearrange(
                                "p (m g) b -> p m g b", g=4))

            def pre1_squash_w2(grp):
                # pre -> squash -> w2 for a 4-batch group, pipelined by T-pair
                with nc.named_scope(f"g{grp}_pre1w2"):
                    w2pn = psW2.tile([128, 4, 16, 2, 4], f32, tag="w2pn")
                    o16 = []
                    for m in range(4):
                        pp = pre_piece(grp, m)
                        o = sbO.tile([128, 2, DC], f16, tag="o16")
                        squash_piece(pp, o, slice(0, 2))
                        o16.append(o)
                        if m >= 1:
                            w2_piece(grp, w2pn, m - 1, o16[m - 1])
                    w2_piece(grp, w2pn, 3, o16[3])
                    w2_gather(grp, w2pn)

            def pre2_piece(grp, m):
                # final pre + squash for T-pair m of one group
                with nc.named_scope(f"pre2_{grp}_{m}"):
                    pp = pre_piece(grp, m)
                    squash_piece(pp, outp32[grp], slice(2 * m, 2 * m + 2))

            def out_dma(grp):
                nc.sync.dma_start(
                    out_drams[grp][:],
                    outp32[grp][:].rearrange("p a b -> p (a b)"))

            # ================= schedule =================
            bup(0, 1); vmm(0, 1)
            keep_warm(W16[:, 0, 0:128]); keep_warm(W16[:, 2, 0:128])
            bup(1, 1); vmm(1, 1)
            keep_warm(WT16[:, 0, 0:128]); keep_warm(WT16[:, 8, 0:128])
            bup(2, 1); vmm(2, 1)
            bup(3, 1); vmm(3, 1)
            pre1_squash_w2(0)
            bup(4, 1); vmm(4, 1)
            bup(5, 1); vmm(5, 1)
            bup(0, 2); vmm(0, 2)
            bup(6, 1); vmm(6, 1)
            bup(1, 2); vmm(1, 2)
            bup(2, 2); vmm(2, 2)
            bup(7, 1); vmm(7, 1)
            bup(3, 2); vmm(3, 2)
            pre1_squash_w2(1)
            bup(4, 2); vmm(4, 2)
            pre2_piece(0, 0); pre2_piece(0, 1)
            bup(5, 2); vmm(5, 2)
            pre2_piece(0, 2); pre2_piece(0, 3)
            out_dma(0)
            bup(6, 2); vmm(6, 2)
            bup(7, 2); vmm(7, 2)
            for m in range(4):
                pre2_piece(1, m)
            out_dma(1)

    nc.compile()
    return nc


def _host_prep(u_vecs, W):
    u_vecs = np.asarray(u_vecs, dtype=np.float32)
    W = np.asarray(W, dtype=np.float32).reshape(D_IN, NC_CAP * DC)
    Wr = W.reshape(D_IN, NC_CAP, DC)

    w16 = np.ascontiguousarray(
        W.reshape(4, 128, NC_CAP * DC).transpose(1, 0, 2)).astype(np.float16)
    # WT packed: [128=(tau,d), 16=(m,g), 512]; capsule n = 8m + 4tau + g
    wt = np.zeros((128, 16, D_IN), dtype=np.float16)
    for m in range(4):
        for g in range(4):
            for tau in range(2):
                n = 8 * m + 4 * tau + g
                wt[64 * tau:64 * tau + 64, 4 * m + g, :] = \
                    Wr[:, n, :].T.astype(np.float16)
    ident = np.eye(128, dtype=np.float16)

    in_maps = []
    for c in range(N_CORES):
        ub = u_vecs[c * B_LOC:(c + 1) * B_LOC]  # [8, 1024, 512] fp32
        u16 = ub.astype(np.float16)
        up = np.ascontiguousarray(
            u16.reshape(B_LOC, 8, 128, D_IN).transpose(0, 2, 1, 3))
        utp = np.ascontiguousarray(
            u16.transpose(0, 2, 1).reshape(B_LOC, 4, 128, N_IN)
            .transpose(0, 2, 1, 3))
        # host iter-0: c is uniform, so outputs0 depends only on column sums
        s = ub.sum(axis=1) / NC_CAP                       # [8, 512] fp32
        pre0 = np.einsum('bk,knd->bnd', s, Wr)
        out0 = pre0 / np.sqrt((pre0 ** 2).sum(-1, keepdims=True) + EPS)
        w20 = np.einsum('bnd,knd->bnk', out0, Wr)         # [8, 32, 512]
        w20t = np.ascontiguousarray(
            w20.transpose(2, 0, 1).reshape(4, 128, B_LOC, NC_CAP)
            .transpose(1, 0, 2, 3)).astype(np.float16)
        in_maps.append({
            "u16": up, "ut16": utp, "w16": w16, "wt16": wt, "w20t": w20t,
            "ident": ident,
        })
    return in_maps


def _unpack_out(rawA, rawB):
    # raw [128, 512] f32; row 32g+c, cols (T, d) -> out[4*grp + c, 4T+g, d]
    out = np.empty((B_LOC, NC_CAP, DC), dtype=np.float32)
    for grp, raw in enumerate((rawA, rawB)):
        r = raw.reshape(4, 32, 8, DC)   # [g, c-slot, T, d]
        for g in range(4):
            for cc in range(4):
                out[4 * grp + cc, 4 * np.arange(8) + g, :] = r[g, cc]
    return out


def kernel(u_vecs, W):
    from concourse.bass_utils import run_bass_kernel_spmd

    if "nc" not in _cached:
        _cached["nc"] = _build_program()
    nc = _cached["nc"]

    in_maps = _host_prep(u_vecs, W)
    res = run_bass_kernel_spmd(nc, in_maps, list(range(N_CORES)))
    out = np.concatenate(
        [_unpack_out(res.results[c]["outA"], res.results[c]["outB"])
         for c in range(N_CORES)], axis=0)
    return out.astype(np.float32)


# revision 13
# speedup vs baseline: 1.0166x; 1.0166x over previous
"""CapsNet dynamic-routing kernel for TRN2, 8 NeuronCores, data-parallel over batch.

Routing math is fully batch-local; the kernel is a per-batch pipeline hidden
under the u-vec DMA stream:

  host: iter-0 (softmax(0) is uniform) -> outputs0, w20 = W @ outputs0 shipped
  chip: per batch b:  b1 = w20 @ u^T -> softmax -> v1 = c1^T u   (as UT/U land)
        per group:    pre1 = v1 @ W -> squash -> w21 = W @ out1  (T-pair pipelined)
        per batch:    b2 = w21 @ u^T -> softmax -> v2
        per group:    pre2 -> squash -> output DMA

Perf notes vs the earlier revision:
  - squash scale = exp(-0.5*ln(|pre|^2)): Ln and Exp live in the same act
    table set (natural_log_exp_and_others), so ONE act-table load for the
    whole kernel instead of 8 Exp<->Sqrt swaps (~10us + serialization).
    |pre|^2 is O(100) here so the reference's 1e-7 eps is numerically inert.
  - PE HAM warm-up: ~4us of junk matmuls at t~1.5us flips the PE clock gate
    to 2.4GHz before real work; keep-warm matmuls anchored to the W16/WT16
    DMA halves bridge the two >3.4us PE-idle windows in the early stream so
    the PE never re-throttles to 1.2GHz.
  - DMA order: W16/WT16 stream *between* early batches (needed by pre1A/w2A
    ~30us in), batches 3..7 stream back-to-back so the tail (which is gated
    by the last batch's arrival) starts as early as possible.
  - PSUM->SBUF copies moved off the scalar queue (DVE), scalar runs only Exp
    softmaxes + Ln/Exp squash scales + w2 gathers.
fp16 operands / fp32 accumulation; inputs host-packed partition-major.
"""

import numpy as np

ROUTINGS = 3
NC_CAP = 32
DC = 64
EPS = 1e-7
N_CORES = 8
B, N_IN, D_IN = 64, 1024, 512
B_LOC = B // N_CORES  # 8

_cached = {}


def _build_program():
    import concourse.bass as bass
    import concourse.tile as tile
    from concourse import bacc, mybir
    from concourse.hw_specs import get_activation_tables

    f16 = mybir.dt.float16
    f32 = mybir.dt.float32
    ADD = mybir.AluOpType.add
    AX = mybir.AxisListType.X
    AF = mybir.ActivationFunctionType

    nc = bacc.Bacc("TRN2", target_bir_lowering=False, debug=False,
                   num_devices=N_CORES)
    act_tabs = list(get_activation_tables(nc.m.arch).keys())
    LNEXP_SET = act_tabs.index("natural_log_exp_and_others")

    # host-packed, SBUF-native layouts (partition dim first, contiguous rows)
    w16_d = nc.dram_tensor("w16", [128, 4, NC_CAP * DC], f16, kind="ExternalInput").ap()
    wt16_d = nc.dram_tensor("wt16", [128, 16, D_IN], f16, kind="ExternalInput").ap()
    w20t_d = nc.dram_tensor("w20t", [128, 4, B_LOC, NC_CAP], f16, kind="ExternalInput").ap()
    ut_d = nc.dram_tensor("ut16", [B_LOC, 128, 4, N_IN], f16, kind="ExternalInput").ap()
    u_d = nc.dram_tensor("u16", [B_LOC, 128, 8, D_IN], f16, kind="ExternalInput").ap()
    ident_d = nc.dram_tensor("ident", [128, 128], f16, kind="ExternalInput").ap()
    outA_d = nc.dram_tensor("outA", [128, 8 * DC], f32, kind="ExternalOutput").ap()
    outB_d = nc.dram_tensor("outB", [128, 8 * DC], f32, kind="ExternalOutput").ap()
    out_drams = [outA_d, outB_d]

    with tile.TileContext(nc) as tc:
        with (
            tc.tile_pool(name="big", bufs=1) as big,
            tc.tile_pool(name="work", bufs=1) as work,
            tc.tile_pool(name="sbE", bufs=2) as sbE,
            tc.tile_pool(name="sbP", bufs=2) as sbP,
            tc.tile_pool(name="sbO", bufs=4) as sbO,
            tc.tile_pool(name="psB", bufs=2, space="PSUM") as psB,
            tc.tile_pool(name="psV", bufs=2, space="PSUM") as psV,
            tc.tile_pool(name="psPre", bufs=2, space="PSUM") as psPre,
            tc.tile_pool(name="psT", bufs=1, space="PSUM") as psT,
            tc.tile_pool(name="psW2", bufs=1, space="PSUM") as psW2,
        ):
            U = big.tile([128, B_LOC, 8, D_IN], f16, tag="U")      # (i%128),(b),(i//128),(k)
            UT = big.tile([128, B_LOC, 4, N_IN], f16, tag="UT")    # (k%128),(b),(k//128),(i)
            W16 = big.tile([128, 4, NC_CAP * DC], f16, tag="W16")  # (k%128),(k//128),(n d)
            WT16 = big.tile([128, 16, D_IN], f16, tag="WT16")      # (tau d),(m g),(k)
            W20T = big.tile([128, 4, B_LOC, NC_CAP], f16, tag="W20T")
            IDENT = work.tile([128, 128], f16, tag="IDENT")
            JUNK = work.tile([128, 256], f16, tag="JUNK")

            # per-group tiles (A: batches 0-3, B: 4-7) to avoid false WARs
            vT = [work.tile([128, 4, 4, NC_CAP], f16, tag=f"vT{g}",
                            name=f"vT{g}") for g in range(2)]
            w2T = [work.tile([128, 4, 4, NC_CAP], f16, tag=f"w2T{g}",
                             name=f"w2T{g}") for g in range(2)]
            c_sb = [work.tile([128, 4, 8, NC_CAP], f16, tag=f"c{g}",
                              name=f"c{g}") for g in range(2)]
            outT = [work.tile([128, 4, 128], f16, tag=f"outT{g}",
                              name=f"outT{g}") for g in range(2)]
            L_sb = [[work.tile([128, 4, 2, 4], f16, tag=f"L{g}_{m}",
                               name=f"L{g}_{m}") for m in range(4)]
                    for g in range(2)]
            outp32 = [work.tile([128, 8, DC], f32, tag=f"outp32_{g}",
                                name=f"outp32_{g}") for g in range(2)]
            z_sb = work.tile([128, B_LOC, 8], f32, tag="z")
            r_sb = work.tile([128, B_LOC, 8], f32, tag="r")

            # ---- single act-table load: Ln+Exp (+Copy) in one set ----
            nc.scalar.add_instruction(mybir.InstLoadActFuncSet(
                name=f"I-{nc.next_id()}", act_func_set_id=LNEXP_SET))
            nc.gpsimd.memset(JUNK[:], 0.5)
            for g in range(2):
                for m in range(4):
                    nc.gpsimd.memset(L_sb[g][m][:], 0.0)

            # ---- PE HAM warm-up: ~5us of junk matmuls from t~1.5us ----
            # (shares the psW2 "w2pn" slot: w2 only starts ~35us in, long
            # after the last keep-warm has retired, so the WAR is free)
            dummy_ps = psW2.tile([128, 4, 16, 2, 4], f32, tag="w2pn")
            with nc.named_scope("ham_warmup"):
                for i in range(48):
                    nc.tensor.matmul(dummy_ps[:, i % 4], JUNK[:, 0:128],
                                     JUNK[:, 0:128],
                                     start=True, stop=True)

            def keep_warm(w):
                # junk weight-load whose source is a just-landed DMA slice:
                # fires the moment the data arrives, bridging a PE-idle
                # window so the HAM clock gate never re-throttles (pure
                # LDWEIGHTS: no PSUM write, no WAR with live accumulators)
                nc.tensor.ldweights(w)

            # ---- DMA queue: order == consumption order of the pipeline ----
            # group-A batches first, then the W tensors (needed by pre1A/w2A
            # ~32us in) split by capsule-half so pieces m0/m1 can start on
            # the first half, then group-B batches (the tail is gated by the
            # last batch, so nothing may come after it)
            nc.sync.dma_start(W20T[:], w20t_d[:])
            for b in range(0, 4):
                nc.sync.dma_start(UT[:, b], ut_d[b])
                nc.sync.dma_start(U[:, b], u_d[b])
            nc.sync.dma_start(W16[:, :, 0:1024], w16_d[:, :, 0:1024])
            nc.sync.dma_start(WT16[:, 0:8], wt16_d[:, 0:8])
            nc.sync.dma_start(W16[:, :, 1024:2048], w16_d[:, :, 1024:2048])
            nc.sync.dma_start(WT16[:, 8:16], wt16_d[:, 8:16])
            nc.sync.dma_start(IDENT[:], ident_d[:])
            for b in range(4, B_LOC):
                nc.sync.dma_start(UT[:, b], ut_d[b])
                nc.sync.dma_start(U[:, b], u_d[b])

            def bup(b, it):
                # b-logits for batch b: [i%128, t, n] = sum_k u^T chunks @ w2T
                src = W20T if it == 1 else w2T[b // 4]
                bl = b if it == 1 else b % 4
                with nc.named_scope(f"i{it}_bup{b}"):
                    b_ps = psB.tile([128, 8, NC_CAP], f32, tag="b_ps")
                    for t in range(8):
                        for j in range(4):
                            nc.tensor.matmul(
                                b_ps[:, t], UT[:, b, j, 128 * t:128 * t + 128],
                                src[:, j, bl, :], start=(j == 0), stop=(j == 3))
                    e_sb = sbE.tile([128, 8, NC_CAP], f16, tag="e_sb")
                    nc.scalar.activation(e_sb[:], b_ps[:], AF.Exp)
                    nc.vector.tensor_reduce(z_sb[:, b], e_sb[:], AX, ADD)
                    nc.vector.reciprocal(r_sb[:, b], z_sb[:, b])
                    nc.vector.tensor_mul(
                        c_sb[b // 4][:, b % 4], e_sb[:],
                        r_sb[:, b].broadcast_to((128, 8, NC_CAP)))

            def vmm(b, it):
                with nc.named_scope(f"i{it}_v{b}"):
                    vT_ps = psV.tile([128, 4, NC_CAP], f32, tag="vT_ps")
                    for j in range(4):
                        for t in range(8):
                            nc.tensor.matmul(
                                vT_ps[:, j], U[:, b, t, 128 * j:128 * j + 128],
                                c_sb[b // 4][:, b % 4, t, :],
                                start=(t == 0), stop=(t == 7))
                    nc.vector.tensor_copy(vT[b // 4][:, :, b % 4, :], vT_ps[:])

            def pre_piece(grp, m):
                # capsules n = 4T+g for T in {2m, 2m+1}; fresh PSUM tile per piece
                pp = psPre.tile([128, 2, DC], f32, tag="pre")
                for tl in range(2):
                    for g in range(4):
                        n = 4 * (2 * m + tl) + g
                        for j in range(4):
                            nc.tensor.matmul(
                                pp[32 * g:32 * g + 4, tl],
                                vT[grp][:, j, :, n],
                                W16[:, j, 64 * n:64 * n + 64],
                                start=(j == 0), stop=(j == 3),
                                tile_position=(0, 32 * g),
                            )
                return pp

            def squash_piece(pp, dst, dsl):
                # dst[:, dsl] = pp / sqrt(|pp|^2), norm over d per capsule;
                # 1/sqrt(x) computed as exp(-0.5*ln(x)) to stay in one table set
                pre_c = sbP.tile([128, 2, DC], f32, tag="pre_c")
                sq2 = sbP.tile([128, 2, DC], f32, tag="sq2")
                nrm = sbP.tile([128, 2], f32, tag="nrm")
                lgn = sbP.tile([128, 2], f32, tag="lgn")
                scl = sbP.tile([128, 2], f32, tag="scl")
                nc.vector.tensor_copy(pre_c[:], pp[:])
                nc.vector.tensor_mul(sq2[:], pre_c[:], pp[:])
                nc.vector.tensor_reduce(nrm[:], sq2[:], AX, ADD)
                nc.scalar.activation(lgn[:], nrm[:], AF.Ln)
                nc.scalar.activation(scl[:], lgn[:], AF.Exp, scale=-0.5)
                nc.vector.tensor_mul(dst[:, dsl], pp[:],
                                     scl[:].broadcast_to((128, 2, DC)))

            def w2_piece(grp, w2pn, m, o16):
                # transpose scaled outputs T-pair m -> outT[(tau d), (g c)],
                # mask into L, then contract d for this piece's capsule pairs
                tp = psT.tile([128, 128], f16, tag="tp")
                nc.tensor.transpose(
                    tp[:], o16[:].rearrange("p a b -> p (a b)"), IDENT[:])
                nc.vector.tensor_copy(outT[grp][:, m], tp[:])
                for tau in range(2):
                    nc.vector.tensor_copy(
                        L_sb[grp][m][64 * tau:64 * tau + 64, :, tau, :],
                        outT[grp][64 * tau:64 * tau + 64, m, :]
                        .rearrange("p (g c) -> p g c", g=4)[:, :, 0:4])
                for p in range(4 * m, 4 * m + 4):
                    for j in range(4):
                        nc.tensor.matmul(
                            w2pn[:, j, p], WT16[:, p, 128 * j:128 * j + 128],
                            L_sb[grp][m][:, p - 4 * m], start=True, stop=True)

            def w2_gather_piece(grp, w2pn, m):
                # capsules n in [8m, 8m+8) = (x g) block of this piece: two
                # strided copies per piece instead of one 8-copy barrier at
                # the end (the first bup of the next iter waits on the last
                # gather, so completion must track the pieces)
                for x in range(2):
                    nc.scalar.copy(
                        w2T[grp][:, :, :, 8 * m + 4 * x:8 * m + 4 * x + 4],
                        w2pn[:, :, 4 * m:4 * m + 4, x, :].rearrange(
                            "p j g b -> p j b g"))

            def squash1_piece(grp, pp, o16, m):
                o = sbO.tile([128, 2, DC], f16, tag="o16")
                squash_piece(pp, o, slice(0, 2))
                o16.append(o)

            def pre2_piece(grp, m):
                # final pre + squash for T-pair m of one group
                with nc.named_scope(f"pre2_{grp}_{m}"):
                    pp = pre_piece(grp, m)
                    squash_piece(pp, outp32[grp], slice(2 * m, 2 * m + 2))

            def out_dma(grp, half):
                nc.sync.dma_start(
                    out_drams[grp][:, 256 * half:256 * half + 256],
                    outp32[grp][:, 4 * half:4 * half + 4].rearrange(
                        "p a b -> p (a b)"))

            # ================= schedule =================
            # program order == per-engine queue order; ordered by when each
            # block's inputs land so nothing DMA-independent blocks
            # DMA-dependent work (in-order queues => head-of-line matters)
            bup(0, 1); vmm(0, 1)
            bup(1, 1); vmm(1, 1)
            bup(2, 1); vmm(2, 1)
            bup(3, 1); vmm(3, 1)
            # W16/WT16 stream in halves now; keep-warms bridge the PE-idle
            # windows, pre1A/w2A pieces chase the halves as they land
            w2pnA = psW2.tile([128, 4, 16, 2, 4], f32, tag="w2pn")
            o16A = []
            keep_warm(W16[:, 0, 0:128])
            pre1A = [pre_piece(0, m) for m in range(2)]
            squash1_piece(0, pre1A[0], o16A, 0)
            keep_warm(WT16[:, 0, 0:128])
            squash1_piece(0, pre1A[1], o16A, 1)
            w2_piece(0, w2pnA, 0, o16A[0]); w2_gather_piece(0, w2pnA, 0)
            keep_warm(W16[:, 0, 1024:1152])
            pre1A.append(pre_piece(0, 2))
            squash1_piece(0, pre1A[2], o16A, 2)
            w2_piece(0, w2pnA, 1, o16A[1]); w2_gather_piece(0, w2pnA, 1)
            keep_warm(WT16[:, 8, 0:128])
            pre1A.append(pre_piece(0, 3))
            squash1_piece(0, pre1A[3], o16A, 3)
            w2_piece(0, w2pnA, 2, o16A[2]); w2_gather_piece(0, w2pnA, 2)
            w2_piece(0, w2pnA, 3, o16A[3]); w2_gather_piece(0, w2pnA, 3)
            # group-B iter-1 as batches land, iter-2 A filling the gaps
            bup(4, 1); vmm(4, 1)
            bup(0, 2); vmm(0, 2)
            bup(5, 1); vmm(5, 1)
            bup(1, 2); vmm(1, 2)
            bup(6, 1); vmm(6, 1)
            bup(2, 2); vmm(2, 2)
            bup(3, 2); vmm(3, 2)
            bup(7, 1); vmm(7, 1)
            # pre2A here: ready before UT7 lands, fills the pre-tail lull;
            # its output DMA goes out while the tail computes
            pre2_piece(0, 0); pre2_piece(0, 1)
            out_dma(0, 0)
            pre2_piece(0, 2); pre2_piece(0, 3)
            out_dma(0, 1)
            # ---- tail: everything below is gated by U7's arrival ----
            w2pnB = psW2.tile([128, 4, 16, 2, 4], f32, tag="w2pn")
            o16B = []
            pre1B = [pre_piece(1, m) for m in range(2)]
            squash1_piece(1, pre1B[0], o16B, 0)
            squash1_piece(1, pre1B[1], o16B, 1)
            w2_piece(1, w2pnB, 0, o16B[0]); w2_gather_piece(1, w2pnB, 0)
            pre1B.append(pre_piece(1, 2))
            squash1_piece(1, pre1B[2], o16B, 2)
            w2_piece(1, w2pnB, 1, o16B[1]); w2_gather_piece(1, w2pnB, 1)
            pre1B.append(pre_piece(1, 3))
            squash1_piece(1, pre1B[3], o16B, 3)
            w2_piece(1, w2pnB, 2, o16B[2]); w2_gather_piece(1, w2pnB, 2)
            w2_piece(1, w2pnB, 3, o16B[3]); w2_gather_piece(1, w2pnB, 3)
            # iter-2 B packed pairwise: bup(b+1) runs while softmax(b) chases
            bup(4, 2); bup(5, 2)
            vmm(4, 2); bup(6, 2)
            vmm(5, 2); bup(7, 2)
            vmm(6, 2); vmm(7, 2)
            pre2_piece(1, 0); pre2_piece(1, 1)
            out_dma(1, 0)
            pre2_piece(1, 2); pre2_piece(1, 3)
            out_dma(1, 1)

    nc.compile()
    return nc


def _host_prep(u_vecs, W):
    u_vecs = np.asarray(u_vecs, dtype=np.float32)
    W = np.asarray(W, dtype=np.float32).reshape(D_IN, NC_CAP * DC)
    Wr = W.reshape(D_IN, NC_CAP, DC)

    w16 = np.ascontiguousarray(
        W.reshape(4, 128, NC_CAP * DC).transpose(1, 0, 2)).astype(np.float16)
    # WT packed: [128=(tau,d), 16=(m,g), 512]; capsule n = 8m + 4tau + g
    wt = np.zeros((128, 16, D_IN), dtype=np.float16)
    for m in range(4):
        for g in range(4):
            for tau in range(2):
                n = 8 * m + 4 * tau + g
                wt[64 * tau:64 * tau + 64, 4 * m + g, :] = \
                    Wr[:, n, :].T.astype(np.float16)
    ident = np.eye(128, dtype=np.float16)

    in_maps = []
    for c in range(N_CORES):
        ub = u_vecs[c * B_LOC:(c + 1) * B_LOC]  # [8, 1024, 512] fp32
        u16 = ub.astype(np.float16)
        up = np.ascontiguousarray(
            u16.reshape(B_LOC, 8, 128, D_IN).transpose(0, 2, 1, 3))
        utp = np.ascontiguousarray(
            u16.transpose(0, 2, 1).reshape(B_LOC, 4, 128, N_IN)
            .transpose(0, 2, 1, 3))
        # host iter-0: c is uniform, so outputs0 depends only on column sums
        s = ub.sum(axis=1) / NC_CAP                       # [8, 512] fp32
        pre0 = np.einsum('bk,knd->bnd', s, Wr)
        out0 = pre0 / np.sqrt((pre0 ** 2).sum(-1, keepdims=True) + EPS)
        w20 = np.einsum('bnd,knd->bnk', out0, Wr)         # [8, 32, 512]
        w20t = np.ascontiguousarray(
            w20.transpose(2, 0, 1).reshape(4, 128, B_LOC, NC_CAP)
            .transpose(1, 0, 2, 3)).astype(np.float16)
        in_maps.append({
            "u16": up, "ut16": utp, "w16": w16, "wt16": wt, "w20t": w20t,
            "ident": ident,
        })
    return in_maps


def _unpack_out(rawA, rawB):
    # raw [128, 512] f32; row 32g+c, cols (T, d) -> out[4*grp + c, 4T+g, d]
    out = np.empty((B_LOC, NC_CAP, DC), dtype=np.float32)
    for grp, raw in enumerate((rawA, rawB)):
        r = raw.reshape(4, 32, 8, DC)   # [g, c-slot, T, d]
        for g in range(4):
            for cc in range(4):
                out[4 * grp + cc, 4 * np.arange(8) + g, :] = r[g, cc]
    return out


def kernel(u_vecs, W):
    from concourse.bass_utils import run_bass_kernel_spmd

    if "nc" not in _cached:
        _cached["nc"] = _build_program()
    nc = _cached["nc"]

    in_maps = _host_prep(u_vecs, W)
    res = run_bass_kernel_spmd(nc, in_maps, list(range(N_CORES)))
    out = np.concatenate(
        [_unpack_out(res.results[c]["outA"], res.results[c]["outB"])
         for c in range(N_CORES)], axis=0)
    return out.astype(np.float32)
